# revision 43
# baseline (speedup 1.0000x reference)
"""Trainium2 Bass kernel for nn_Attention (GroupNorm + MHA + proj + residual).

Reference (per batch b of 16, C=512, T=32*32=1024, 8 heads, head_dim 64):
  xr   = x.reshape(B, C, T)
  h    = group_norm(xr, 32 groups of 16 ch x T)  * norm_w + norm_b
  qkv  = qkv_w @ h + qkv_b          (per-head contiguous [q;k;v] chunks)
  S    = (q/8^.5)^T (k/8^.5)        per head-batch  [T, T]
  P    = softmax(S)
  o    = P @ v^T  -> [ch, T];  out = proj_w @ o + proj_b + xr

Sharding: pure data-parallel over batch: 2 batches per core x 8 cores.

v2 design (vs baseline):
  - all matmul operands bf16 (f32r weight loads kept the PE array at ~50%
    duty -> HAM clock-gate throttled it to 1.2GHz for half the kernel)
  - S matmuls row-paired: heads (2p, 2p+1) live in partitions 0-63/64-127
    of shared q2/k2 tiles; the two K=64 matmuls run concurrently
  - k-bias dropped (adds a per-query constant to logits -> cancels in
    softmax); q-bias applied as one per-pair DVE op
  - per-head Z chain: o_sb copy -> reciprocal_approx_fast on the Z row
    -> gpsimd partition_broadcast -> gpsimd multiply (no SBUF-SBUF DMAs)
  - PSUM: pool A 3 x [128,1024] (qk / S / proj rotation)
          pool B 2 x [128,512]  (v tiles / O accumulation per t-half)
"""
import math
import numpy as np

B, C, T, NH, HD = 16, 512, 1024, 8, 64
NCORES = 8
BPC = B // NCORES          # batches per core
CT = C // 128              # channel tiles (4)
ST = T // 128              # s tiles (8)
TH = T // 512              # t halves (2)
NP = NH // 2               # head pairs (4)
EPS = 1e-5

_CACHE = {}


def _build_nc(debug=False):
    import concourse.bass as bass
    from concourse import bacc
    import concourse.tile as tile
    from concourse import mybir
    from contextlib import ExitStack

    F32 = mybir.dt.float32
    BF16 = mybir.dt.bfloat16
    AF = mybir.ActivationFunctionType
    OP = mybir.AluOpType

    nc = bacc.Bacc(trn_type="TRN2", name="attn")

    x = nc.dram_tensor("x", [BPC, C, T], F32, kind="ExternalInput")
    wqk = nc.dram_tensor("wqk", [C, 2 * C], BF16, kind="ExternalInput")
    bqk = nc.dram_tensor("bqk", [128, NP], F32, kind="ExternalInput")
    wv = nc.dram_tensor("wv", [C, C], BF16, kind="ExternalInput")
    wp = nc.dram_tensor("wp", [C, C], BF16, kind="ExternalInput")
    pb = nc.dram_tensor("pb", [128, CT], F32, kind="ExternalInput")
    nw = nc.dram_tensor("nw", [128, CT], F32, kind="ExternalInput")
    nb = nc.dram_tensor("nb", [128, CT], F32, kind="ExternalInput")
    em = nc.dram_tensor("em", [8, 128], F32, kind="ExternalInput")
    gm = nc.dram_tensor("gm", [128, 8], F32, kind="ExternalInput")
    y = nc.dram_tensor("y", [BPC, C, T], F32, kind="ExternalOutput")
    if debug:
        dbg_h = nc.dram_tensor("dbg_h", [128, CT, T], BF16, kind="ExternalOutput")
        dbg_q2 = nc.dram_tensor("dbg_q2", [128, T], BF16, kind="ExternalOutput")
        dbg_k2 = nc.dram_tensor("dbg_k2", [128, T], BF16, kind="ExternalOutput")
        dbg_v = nc.dram_tensor("dbg_v", [128, NH, HD + 1], BF16, kind="ExternalOutput")
        dbg_P = nc.dram_tensor("dbg_P", [2, 128, T], BF16, kind="ExternalOutput")
        dbg_o = nc.dram_tensor("dbg_o", [2, HD + 1, T], F32, kind="ExternalOutput")
        dbg_r = nc.dram_tensor("dbg_r", [2, 1, T], F32, kind="ExternalOutput")
        dbg_a = nc.dram_tensor("dbg_a", [128, CT, T], BF16, kind="ExternalOutput")

    with tile.TileContext(nc) as tc, ExitStack() as ctx:
        consts = ctx.enter_context(tc.tile_pool(name="consts", bufs=1))
        xpool = ctx.enter_context(tc.tile_pool(name="xpool", bufs=2))
        hpool = ctx.enter_context(tc.tile_pool(name="hpool", bufs=2))
        qkpool = ctx.enter_context(tc.tile_pool(name="qkpool", bufs=2))
        vpool = ctx.enter_context(tc.tile_pool(name="vpool", bufs=2 * ST))
        ppool = ctx.enter_context(tc.tile_pool(name="ppool", bufs=20))
        opool = ctx.enter_context(tc.tile_pool(name="opool", bufs=2))
        apool = ctx.enter_context(tc.tile_pool(name="apool", bufs=2))
        ypool = ctx.enter_context(tc.tile_pool(name="ypool", bufs=2))
        rpool = ctx.enter_context(tc.tile_pool(name="rpool", bufs=2))
        rbpool = ctx.enter_context(tc.tile_pool(name="rbpool", bufs=2))
        tmp = ctx.enter_context(tc.tile_pool(name="tmp", bufs=2))
        psA = ctx.enter_context(tc.tile_pool(name="psA", bufs=6, space="PSUM"))
        psB = ctx.enter_context(tc.tile_pool(name="psB", bufs=2, space="PSUM"))

        # ---- x loads first: they gate group-norm; const DMAs queue
        #      behind them on the sync engine's in-order DMA-issue stream
        x_list = [None, None]

        def emit_x_load(b):
            x_s = xpool.tile([128, CT, T], F32, tag="x", name=f"x{b}")
            for j in range(CT):
                nc.sync.dma_start(
                    out=x_s[:, j, :], in_=x.ap()[b, 128 * j:128 * (j + 1), :]
                )
            x_list[b] = x_s

        emit_x_load(0)
        emit_x_load(1)

        # ---- constants: small ones first (group-norm needs em/gm/nw/nb
        #      early; the big weights queue after on the DMA-issue stream)
        em_s = consts.tile([8, 128], F32)
        nc.sync.dma_start(out=em_s, in_=em.ap())
        gm_s = consts.tile([128, 8], F32)
        nc.sync.dma_start(out=gm_s, in_=gm.ap())
        nw_s = consts.tile([128, CT], F32)
        nc.sync.dma_start(out=nw_s, in_=nw.ap())
        nb_s = consts.tile([128, CT], F32)
        nc.sync.dma_start(out=nb_s, in_=nb.ap())
        bqk_s = consts.tile([128, NP], F32)
        nc.sync.dma_start(out=bqk_s, in_=bqk.ap())
        pb_s = consts.tile([128, CT], F32)
        nc.sync.dma_start(out=pb_s, in_=pb.ap())
        wqk_s = consts.tile([128, CT, 2 * C], BF16)
        nc.sync.dma_start(out=wqk_s, in_=wqk.ap().rearrange("(j p) n -> p j n", p=128))
        wv_s = consts.tile([128, CT, C], BF16)
        nc.sync.dma_start(out=wv_s, in_=wv.ap().rearrange("(j p) n -> p j n", p=128))
        wp_s = consts.tile([128, CT, C], BF16)
        nc.sync.dma_start(out=wp_s, in_=wp.ap().rearrange("(j p) n -> p j n", p=128))
        eps_s = consts.tile([8, 1], F32)
        nc.vector.memset(eps_s, EPS)

        # ---- per-batch prologue: group-norm stats, h (bf16) ----
        h_list = [None, None]

        def emit_gn_h(b):
            x_s = x_list[b]
            gs = psB.tile([8, 8], F32, tag="B", name=f"gs{b}")
            for j in range(CT):
                st = tmp.tile([128, 2, 6], F32, tag="st")
                nc.vector.bn_stats(out=st[:, 0, :], in_=x_s[:, j, 0:512])
                nc.vector.bn_stats(out=st[:, 1, :], in_=x_s[:, j, 512:1024])
                mv = tmp.tile([128, 2], F32, tag="mv")
                nc.vector.bn_aggr(out=mv, in_=st)
                s2 = tmp.tile([128, 2], F32, tag="s2")
                nc.vector.tensor_copy(out=s2[:, 0:1], in_=mv[:, 0:1])
                # E[x^2] = mean*mean + var
                nc.vector.scalar_tensor_tensor(
                    out=s2[:, 1:2], in0=mv[:, 0:1], scalar=mv[:, 0:1],
                    in1=mv[:, 1:2], op0=OP.mult, op1=OP.add,
                )
                nc.tensor.matmul(gs[:, j:j + 1], gm_s, s2[:, 0:1],
                                 start=True, stop=True)
                nc.tensor.matmul(gs[:, 4 + j:5 + j], gm_s, s2[:, 1:2],
                                 start=True, stop=True)

            gsb = tmp.tile([8, 8], F32, tag="gsb")
            nc.vector.tensor_copy(out=gsb, in_=gs)
            msq = tmp.tile([8, 4], F32, tag="msq")
            nc.vector.tensor_mul(out=msq, in0=gsb[:, 0:4], in1=gsb[:, 0:4])
            varg = tmp.tile([8, 4], F32, tag="varg")
            nc.vector.tensor_tensor(out=varg, in0=gsb[:, 4:8], in1=msq,
                                    op=OP.subtract)
            lng = tmp.tile([8, 4], F32, tag="lng")
            nc.scalar.activation(out=lng, in_=varg, func=AF.Ln, bias=eps_s)
            rstd = tmp.tile([8, 4], F32, tag="rstd")
            nc.scalar.activation(out=rstd, in_=lng, func=AF.Exp, scale=-0.5)
            mr = tmp.tile([8, 8], F32, tag="mr")
            nc.vector.tensor_copy(out=mr[:, 0:4], in_=gsb[:, 0:4])
            nc.vector.tensor_copy(out=mr[:, 4:8], in_=rstd)
            mexp = psB.tile([128, 8], F32, tag="B", name=f"mexp{b}")
            nc.tensor.matmul(mexp, em_s, mr, start=True, stop=True)
            scale_c = tmp.tile([128, CT], F32, tag="scale_c")
            nc.vector.tensor_mul(out=scale_c, in0=mexp[:, 4:8], in1=nw_s)
            mscl = tmp.tile([128, CT], F32, tag="mscl")
            nc.vector.tensor_mul(out=mscl, in0=mexp[:, 0:4], in1=scale_c)
            bias_c = tmp.tile([128, CT], F32, tag="bias_c")
            nc.vector.tensor_tensor(out=bias_c, in0=nb_s, in1=mscl,
                                    op=OP.subtract)

            h_s = hpool.tile([128, CT, T], BF16, tag="h", name=f"h{b}")
            for j in range(CT):
                nc.vector.tensor_scalar(
                    out=h_s[:, j, :], in0=x_s[:, j, :],
                    scalar1=scale_c[:, j:j + 1], scalar2=bias_c[:, j:j + 1],
                    op0=OP.mult, op1=OP.add,
                )
            h_list[b] = h_s

        # ---- emission helpers ----
        v_tiles = [[None] * ST for _ in range(BPC)]
        a_tiles = [[None] * CT for _ in range(BPC)]
        qk_tiles = {}

        # v: one [128(s),512(vdims)] psum tile per s-tile; N=512 = one bank.
        def emit_v_group(b, i):
            h_s = h_list[b]
            pv = psB.tile([128, 512], F32, tag="B", name=f"pv{b}_{i}")
            for kc in range(CT):
                nc.tensor.matmul(
                    pv,
                    h_s[:, kc, i * 128:(i + 1) * 128],
                    wv_s[:, kc, :],
                    start=(kc == 0), stop=(kc == CT - 1),
                )
            v_s = vpool.tile([128, NH, HD + 1], BF16, tag="v",
                             name=f"v{b}_{i}")
            nc.vector.memset(v_s[:, :, HD:HD + 1], 1.0)
            nc.vector.tensor_copy(
                out=v_s[:, :, 0:HD],
                in_=pv.rearrange("p (h d) -> p h d", d=HD),
            )
            v_tiles[b][i] = v_s
            if debug and b == 0 and i == 0:
                nc.sync.dma_start(out=dbg_v.ap(), in_=v_s)

        def emit_qk_half(b, p, which, th):
            """One t-half of q or k for pair p: 4 matmuls into a 1-bank
            psA tile + one DVE op into the bf16 q2/k2 tile."""
            h_s = h_list[b]
            off = p * 256 + (0 if which == "q" else 128)
            pq = psA.tile([128, 512], F32, tag="A",
                          name=f"p{which}{b}_{p}_{th}")
            for kc in range(CT):
                nc.tensor.matmul(
                    pq,
                    wqk_s[:, kc, off:off + 128],
                    h_s[:, kc, th * 512:(th + 1) * 512],
                    start=(kc == 0), stop=(kc == CT - 1),
                )
            if (b, p, which) not in qk_tiles:
                qk_tiles[(b, p, which)] = qkpool.tile(
                    [128, T], BF16, tag=which + "2", name=f"{which}2_{b}_{p}")
            t2 = qk_tiles[(b, p, which)]
            if which == "q":
                nc.vector.tensor_scalar_add(
                    out=t2[:, th * 512:(th + 1) * 512], in0=pq,
                    scalar1=bqk_s[:, p:p + 1])
            else:
                nc.vector.tensor_copy(
                    out=t2[:, th * 512:(th + 1) * 512], in_=pq)

        def emit_qk(b, p):
            for th in range(TH):
                emit_qk_half(b, p, "q", th)
            for th in range(TH):
                emit_qk_half(b, p, "k", th)
            if debug and b == 0 and p == 0:
                nc.sync.dma_start(out=dbg_h.ap(), in_=h_list[b])
                nc.sync.dma_start(out=dbg_q2.ap(), in_=qk_tiles[(b, p, "q")])
                nc.sync.dma_start(out=dbg_k2.ap(), in_=qk_tiles[(b, p, "k")])

        def emit_pair_attention(b, p, interleave=(), self_tail=False):
            """S + exp for pair p, with deferred emissions (previous pair's
            O chains, v groups of the other batch, proj pieces) interleaved
            between s-tiles. Returns closures for this pair's O chains.
            self_tail=True (last pair): head 0's O matmuls are interleaved
            into this pair's own s-loop and head 1's follow immediately, so
            nothing is left pending."""
            q2 = qk_tiles[(b, p, "q")]
            k2 = qk_tiles[(b, p, "k")]
            p_tiles = [[None] * ST, [None] * ST]  # per head-half
            inter = list(interleave)
            st_pO = [None, None]
            st_osb = [None]
            for i in range(ST):
                P0 = ppool.tile([128, T], BF16, tag="P", name=f"P{b}_{p}_{i}a")
                P1 = ppool.tile([128, T], BF16, tag="P", name=f"P{b}_{p}_{i}b")
                # 1-bank score tiles + alternating row groups (head 0 rows
                # 0-63, head 1 rows 64-127): deep psA rotation lets the S
                # matmuls queue ahead and co-issue in the PE array
                for th in range(TH):
                    pS0 = psA.tile([128, 512], F32, tag="A",
                                   name=f"pS{b}_{p}_{i}a{th}")
                    pS1 = psA.tile([128, 512], F32, tag="A",
                                   name=f"pS{b}_{p}_{i}b{th}")
                    nc.tensor.matmul(
                        pS0,
                        k2[0:64, i * 128:(i + 1) * 128],
                        q2[0:64, th * 512:(th + 1) * 512],
                        start=True, stop=True,
                    )
                    nc.tensor.matmul(
                        pS1,
                        k2[64:128, i * 128:(i + 1) * 128],
                        q2[64:128, th * 512:(th + 1) * 512],
                        start=True, stop=True,
                    )
                    nc.scalar.activation(out=P0[:, th * 512:(th + 1) * 512],
                                         in_=pS0, func=AF.Exp, scale=0.125)
                    nc.scalar.activation(out=P1[:, th * 512:(th + 1) * 512],
                                         in_=pS1, func=AF.Exp, scale=0.125)
                p_tiles[0][i] = P0
                p_tiles[1][i] = P1
                if debug and b == 0 and p == 0 and i == 0:
                    nc.sync.dma_start(out=dbg_P.ap()[0], in_=P0)
                    nc.sync.dma_start(out=dbg_P.ap()[1], in_=P1)
                # one deferred emission per s-tile keeps their waits spread
                # across the exp stream instead of bunching at the boundary
                if self_tail:
                    # drain ALL deferred psB users before the self-tail's
                    # persistent pO tiles take both psB slots (else deadlock)
                    npop = -(-len(inter) // max(1, 4 - i)) if i < 4 else len(inter)
                    for _ in range(npop):
                        inter.pop(0)()
                    if i >= 4:
                        ii = i - 4
                        if ii == 0:
                            st_osb[0] = opool.tile(
                                [HD + 1, T], F32, tag="o", name=f"o{b}_{2 * p}")
                            for th in range(TH):
                                st_pO[th] = psB.tile(
                                    [HD + 1, 512], F32, tag="B",
                                    name=f"pO{b}_{2 * p}_{th}")
                        for th in range(TH):
                            nc.tensor.matmul(
                                st_pO[th],
                                v_tiles[b][ii][:, 2 * p, :],
                                p_tiles[0][ii][:, th * 512:(th + 1) * 512],
                                start=(ii == 0), stop=(ii == ST - 1),
                            )
                else:
                    # front-load deferred emissions (2/s-tile early on) so
                    # s-tile 7 and the pair boundary stay clear of them
                    npop = 2 if i < 5 else (1 if i < ST - 1 else 0)
                    for _ in range(min(npop, len(inter))):
                        inter.pop(0)()
            for e in inter:
                e()

            def make_o_group(half, th, chain=True):
                def emit():
                    j = 2 * p + half
                    o_sb = o_sbs[half]
                    pO = psB.tile([HD + 1, 512], F32, tag="B",
                                  name=f"pO{b}_{j}_{th}")
                    for i in range(ST):
                        nc.tensor.matmul(
                            pO,
                            v_tiles[b][i][:, j, :],
                            p_tiles[half][i][:, th * 512:(th + 1) * 512],
                            start=(i == 0), stop=(i == ST - 1),
                        )
                    nc.vector.tensor_copy(
                        out=o_sb[:, th * 512:(th + 1) * 512], in_=pO)
                    if chain and th == TH - 1:
                        emit_z_chain(half, o_sb)
                return emit

            o_sbs = [None, None]

            def alloc_osb():
                for half in range(2):
                    o_sbs[half] = opool.tile(
                        [HD + 1, T], F32, tag="o", name=f"o{b}_{2 * p + half}")

            def emit_z_chain(half, o_sb):
                j = 2 * p + half
                # partition_broadcast / custom-DVE ops read the tile's
                # absolute partition 0 (AP base offsets ignored) -- move the
                # Z row to a base-0 tile via DMA first.
                z0 = rpool.tile([1, T], F32, tag="z0", name=f"z0{b}_{j}")
                nc.sync.dma_start(out=z0, in_=o_sb[HD:HD + 1, :])
                r_s = rpool.tile([1, T], F32, tag="r", name=f"r{b}_{j}")
                nc.vector.reciprocal_approx_fast(out=r_s, in_=z0)
                rb_s = rbpool.tile([HD, T], F32, tag="rb", name=f"rb{b}_{j}")
                nc.gpsimd.partition_broadcast(out_ap=rb_s, in_ap=r_s)
                if debug and b == 0 and j < 2:
                    nc.sync.dma_start(out=dbg_o.ap()[j], in_=o_sb)
                    nc.sync.dma_start(out=dbg_r.ap()[j], in_=r_s)
                if a_tiles[b][j // 2] is None:
                    a_tiles[b][j // 2] = apool.tile(
                        [128, T], BF16, tag=f"a{j // 2}", name=f"a{b}_{j // 2}")
                po2 = (j % 2) * 64
                nc.vector.tensor_mul(
                    out=a_tiles[b][j // 2][po2:po2 + 64, :],
                    in0=o_sb[0:HD, :], in1=rb_s,
                )

            if self_tail:
                # finish head 0: remaining accumulation steps + copies
                for ii in range(4, ST):
                    for th in range(TH):
                        nc.tensor.matmul(
                            st_pO[th],
                            v_tiles[b][ii][:, 2 * p, :],
                            p_tiles[0][ii][:, th * 512:(th + 1) * 512],
                            start=(ii == 0), stop=(ii == ST - 1),
                        )
                o_sbs[0] = st_osb[0]
                for th in range(TH):
                    nc.vector.tensor_copy(
                        out=st_osb[0][:, th * 512:(th + 1) * 512],
                        in_=st_pO[th])
                # tail: run the two heads' Z chains with maximal overlap,
                # and start this batch's proj (kc 0..2) under them.
                z0a = rpool.tile([1, T], F32, tag="z0", name=f"z0{b}_{2 * p}")
                nc.sync.dma_start(out=z0a, in_=st_osb[0][HD:HD + 1, :])
                ra = rpool.tile([1, T], F32, tag="r", name=f"r{b}_{2 * p}")
                nc.vector.reciprocal_approx_fast(out=ra, in_=z0a)
                o_sbs[1] = opool.tile([HD + 1, T], F32, tag="o",
                                      name=f"o{b}_{2 * p + 1}")
                for th in range(TH):
                    make_o_group(1, th, chain=False)()
                for jo in range(CT - 1):
                    emit_proj_start(b, jo, 3)
                z0b = rpool.tile([1, T], F32, tag="z0",
                                 name=f"z0{b}_{2 * p + 1}")
                nc.sync.dma_start(out=z0b, in_=o_sbs[1][HD:HD + 1, :])
                rb_ = rpool.tile([1, T], F32, tag="r", name=f"r{b}_{2 * p + 1}")
                nc.vector.reciprocal_approx_fast(out=rb_, in_=z0b)
                rba = rbpool.tile([HD, T], F32, tag="rb", name=f"rb{b}_{2 * p}")
                nc.gpsimd.partition_broadcast(out_ap=rba, in_ap=ra)
                rbb = rbpool.tile([HD, T], F32, tag="rb",
                                  name=f"rb{b}_{2 * p + 1}")
                nc.gpsimd.partition_broadcast(out_ap=rbb, in_ap=rb_)
                if a_tiles[b][p] is None:
                    a_tiles[b][p] = apool.tile(
                        [128, T], BF16, tag=f"a{p}", name=f"a{b}_{p}")
                nc.vector.tensor_mul(out=a_tiles[b][p][0:HD, :],
                                     in0=o_sbs[0][0:HD, :], in1=rba)
                nc.vector.tensor_mul(out=a_tiles[b][p][HD:128, :],
                                     in0=o_sbs[1][0:HD, :], in1=rbb)
                for jo in range(CT - 1):
                    emit_proj_finish(b, jo, 3)
                emit_proj(b, CT - 1)
                return []

            groups = [alloc_osb]
            for half in range(2):
                for th in range(TH):
                    groups.append(make_o_group(half, th))
            return groups

        pp_tiles = {}

        def emit_proj_start(b, jo, nkc):
            for th in range(TH):
                pp = psA.tile([128, 512], F32, tag="A", name=f"pp{b}_{jo}_{th}")
                pp_tiles[(b, jo, th)] = pp
                for kc in range(nkc):
                    nc.tensor.matmul(
                        pp,
                        wp_s[:, kc, jo * 128:(jo + 1) * 128],
                        a_tiles[b][kc][:, th * 512:(th + 1) * 512],
                        start=(kc == 0), stop=(kc == CT - 1),
                    )

        def emit_proj_finish(b, jo, nkc):
            y_s = ypool.tile([128, T], F32, tag="y", name=f"y{b}_{jo}")
            for th in range(TH):
                pp = pp_tiles[(b, jo, th)]
                for kc in range(nkc, CT):
                    nc.tensor.matmul(
                        pp,
                        wp_s[:, kc, jo * 128:(jo + 1) * 128],
                        a_tiles[b][kc][:, th * 512:(th + 1) * 512],
                        start=(kc == 0), stop=(kc == CT - 1),
                    )
                nc.vector.scalar_tensor_tensor(
                    out=y_s[:, th * 512:(th + 1) * 512], in0=pp,
                    scalar=pb_s[:, jo:jo + 1],
                    in1=x_list[b][:, jo, th * 512:(th + 1) * 512],
                    op0=OP.add, op1=OP.add,
                )
            nc.sync.dma_start(
                out=y.ap()[b, 128 * jo:128 * (jo + 1), :], in_=y_s
            )

        def emit_proj(b, jo):
            if debug and b == 0 and jo == 0:
                for kc in range(CT):
                    nc.sync.dma_start(out=dbg_a.ap()[:, kc, :],
                                      in_=a_tiles[0][kc])
            emit_proj_start(b, jo, CT)
            emit_proj_finish(b, jo, CT)

        # ---- schedule ----
        # head: batch-0 gn/h -> first qk immediately (v groups and batch-1
        # gn/h ride behind it). Each pair's O chains AND the next pair's
        # qk generation are interleaved into the s-loop so pair boundaries
        # have no serial block; the last pair self-interleaves its O work
        # and proj so nothing trails but the final y writes.
        emit_gn_h(0)
        emit_qk(0, 0)
        emit_gn_h(1)
        for i in range(ST):
            emit_v_group(0, i)
        vb1 = [(lambda b=1, i=i: emit_v_group(b, i)) for i in range(ST)]

        def weave(pend_, qks, extras):
            # alternate previous-pair O groups with next-pair qk halves;
            # the front-loaded pop schedule drains all of it by s-tile 6.
            out, a, q = [], list(pend_), list(qks)
            while a or q:
                if a:
                    out.append(a.pop(0))
                if q:
                    out.append(q.pop(0))
            return out + list(extras)

        def qk_closures(b, p):
            return [(lambda b_=b, p_=p, w_=w, t_=t: emit_qk_half(b_, p_, w_, t_))
                    for w in ("q", "k") for t in range(TH)]

        pend = []
        for p in range(NP):
            qks = qk_closures(0, p + 1) if p < NP - 1 else qk_closures(1, 0)
            extras = (vb1[0:3] if p == 1 else vb1[3:6] if p == 2
                      else vb1[6:8] if p == 3 else [])
            pend = emit_pair_attention(0, p,
                                       interleave=weave(pend, qks, extras))
        # batch 1 pairs with batch-0 proj interleaved
        for p in range(NP):
            qks = qk_closures(1, p + 1) if p < NP - 1 else []
            extras = [(lambda jo=p: emit_proj(0, jo))]
            pend = emit_pair_attention(1, p,
                                       interleave=weave(pend, qks, extras),
                                       self_tail=(p == NP - 1))

    nc.finalize()
    return nc


def _prepack(qkv_w, qkv_b, proj_w, proj_b, norm_w, norm_b):
    """Host-side weight packing (numpy; matmul operands cast to bf16)."""
    import ml_dtypes

    BF = ml_dtypes.bfloat16
    wqk = np.empty((C, 2 * C), dtype=np.float32)
    bqk = np.empty((128, NP), dtype=np.float32)
    wv = np.empty((C, C), dtype=np.float32)
    bv = np.empty((C,), dtype=np.float32)
    for h in range(NH):
        base = 3 * HD * h  # 192h
        p, half = h // 2, h % 2
        # pair-major: [q(2p)|q(2p+1)] then [k(2p)|k(2p+1)], 256 cols/pair
        wqk[:, 256 * p + 64 * half: 256 * p + 64 * (half + 1)] = \
            qkv_w[base:base + HD, :].T
        wqk[:, 256 * p + 128 + 64 * half: 256 * p + 128 + 64 * (half + 1)] = \
            qkv_w[base + HD:base + 2 * HD, :].T
        bqk[64 * half:64 * (half + 1), p] = qkv_b[base:base + HD]
        wv[:, HD * h:HD * (h + 1)] = qkv_w[base + 128:base + 192, :].T
        bv[HD * h:HD * (h + 1)] = qkv_b[base + 128:base + 192]
    wp = np.ascontiguousarray(proj_w.T)
    pbv = proj_b + proj_w @ bv
    pb = np.ascontiguousarray(pbv.reshape(CT, 128).T)
    nw = np.ascontiguousarray(norm_w.reshape(CT, 128).T)
    nb = np.ascontiguousarray(norm_b.reshape(CT, 128).T)
    em = np.zeros((8, 128), dtype=np.float32)
    gm = np.zeros((128, 8), dtype=np.float32)
    for p in range(128):
        em[p // 16, p] = 1.0
        gm[p, p // 16] = 1.0 / 16.0  # bn_aggr outputs are already per-T means
    return dict(wqk=wqk.astype(BF), bqk=bqk, wv=wv.astype(BF),
                wp=wp.astype(BF), pb=pb, nw=nw, nb=nb, em=em, gm=gm)


def kernel(**inputs):
    from concourse.bass_utils import run_bass_kernel_spmd

    x = np.ascontiguousarray(np.asarray(inputs["x"], dtype=np.float32))
    assert x.shape == (B, C, 32, 32)
    nh = int(np.asarray(inputs["num_heads"]))
    assert nh == NH, f"kernel hardcodes num_heads={NH}, got {nh}"

    packed = _prepack(
        np.asarray(inputs["qkv_w"], dtype=np.float32),
        np.asarray(inputs["qkv_b"], dtype=np.float32),
        np.asarray(inputs["proj_w"], dtype=np.float32),
        np.asarray(inputs["proj_b"], dtype=np.float32),
        np.asarray(inputs["norm_w"], dtype=np.float32),
        np.asarray(inputs["norm_b"], dtype=np.float32),
    )

    if "nc" not in _CACHE:
        _CACHE["nc"] = _build_nc()
    nc = _CACHE["nc"]

    xr = x.reshape(B, C, T)
    in_maps = []
    for c in range(NCORES):
        m = dict(packed)
        m["x"] = np.ascontiguousarray(xr[c * BPC:(c + 1) * BPC])
        in_maps.append(m)

    # Execute twice and compare: guards against a rare first-execution
    # flake observed after a fresh NEFF load. Extra exec costs ~ms.
    def run_once():
        res = run_bass_kernel_spmd(nc, in_maps, core_ids=list(range(NCORES)))
        return np.concatenate(
            [res.results[c]["y"] for c in range(NCORES)], axis=0
        )

    out1 = run_once()
    out2 = run_once()
    if not np.array_equal(out1, out2):
        out3 = run_once()
        out1 = out3 if np.array_equal(out2, out3) else out2
        if np.array_equal(out2, out3):
            out1 = out2
    return out1.reshape(B, C, 32, 32).astype(np.float32)


# revision 45
# speedup vs baseline: 1.0435x; 1.0435x over previous
"""Trainium2 Bass kernel for nn_Attention (GroupNorm + MHA + proj + residual).

Reference (per batch b of 16, C=512, T=32*32=1024, 8 heads, head_dim 64):
  xr   = x.reshape(B, C, T)
  h    = group_norm(xr, 32 groups of 16 ch x T)  * norm_w + norm_b
  qkv  = qkv_w @ h + qkv_b          (per-head contiguous [q;k;v] chunks)
  S    = (q/8^.5)^T (k/8^.5)        per head-batch  [T, T]
  P    = softmax(S)
  o    = P @ v^T  -> [ch, T];  out = proj_w @ o + proj_b + xr

Sharding: pure data-parallel over batch: 2 batches per core x 8 cores.

v2 design (vs baseline):
  - all matmul operands bf16 (f32r weight loads kept the PE array at ~50%
    duty -> HAM clock-gate throttled it to 1.2GHz for half the kernel)
  - S matmuls row-paired: heads (2p, 2p+1) live in partitions 0-63/64-127
    of shared q2/k2 tiles; the two K=64 matmuls run concurrently
  - k-bias dropped (adds a per-query constant to logits -> cancels in
    softmax); q-bias applied as one per-pair DVE op
  - per-head Z chain: o_sb copy -> reciprocal_approx_fast on the Z row
    -> gpsimd partition_broadcast -> gpsimd multiply (no SBUF-SBUF DMAs)
  - PSUM: pool A 3 x [128,1024] (qk / S / proj rotation)
          pool B 2 x [128,512]  (v tiles / O accumulation per t-half)
"""
import math
import numpy as np

B, C, T, NH, HD = 16, 512, 1024, 8, 64
NCORES = 8
BPC = B // NCORES          # batches per core
CT = C // 128              # channel tiles (4)
ST = T // 128              # s tiles (8)
TH = T // 512              # t halves (2)
NP = NH // 2               # head pairs (4)
EPS = 1e-5

_CACHE = {}


def _build_nc(debug=False):
    import concourse.bass as bass
    from concourse import bacc
    import concourse.tile as tile
    from concourse import mybir
    from contextlib import ExitStack

    F32 = mybir.dt.float32
    BF16 = mybir.dt.bfloat16
    AF = mybir.ActivationFunctionType
    OP = mybir.AluOpType

    nc = bacc.Bacc(trn_type="TRN2", name="attn")

    x = nc.dram_tensor("x", [BPC, C, T], F32, kind="ExternalInput")
    wqk = nc.dram_tensor("wqk", [C, 2 * C], BF16, kind="ExternalInput")
    bqk = nc.dram_tensor("bqk", [128, NP], F32, kind="ExternalInput")
    wv = nc.dram_tensor("wv", [C, C], BF16, kind="ExternalInput")
    wp = nc.dram_tensor("wp", [C, C], BF16, kind="ExternalInput")
    pb = nc.dram_tensor("pb", [128, CT], F32, kind="ExternalInput")
    nw = nc.dram_tensor("nw", [128, CT], F32, kind="ExternalInput")
    nb = nc.dram_tensor("nb", [128, CT], F32, kind="ExternalInput")
    em = nc.dram_tensor("em", [8, 128], F32, kind="ExternalInput")
    gm = nc.dram_tensor("gm", [128, 8], F32, kind="ExternalInput")
    y = nc.dram_tensor("y", [BPC, C, T], F32, kind="ExternalOutput")
    if debug:
        dbg_h = nc.dram_tensor("dbg_h", [128, CT, T], BF16, kind="ExternalOutput")
        dbg_q2 = nc.dram_tensor("dbg_q2", [128, T], BF16, kind="ExternalOutput")
        dbg_k2 = nc.dram_tensor("dbg_k2", [128, T], BF16, kind="ExternalOutput")
        dbg_v = nc.dram_tensor("dbg_v", [128, NH, HD + 1], BF16, kind="ExternalOutput")
        dbg_P = nc.dram_tensor("dbg_P", [2, 128, T], BF16, kind="ExternalOutput")
        dbg_o = nc.dram_tensor("dbg_o", [2, HD + 1, T], F32, kind="ExternalOutput")
        dbg_r = nc.dram_tensor("dbg_r", [2, 1, T], F32, kind="ExternalOutput")
        dbg_a = nc.dram_tensor("dbg_a", [128, CT, T], BF16, kind="ExternalOutput")

    with tile.TileContext(nc) as tc, ExitStack() as ctx:
        consts = ctx.enter_context(tc.tile_pool(name="consts", bufs=1))
        xpool = ctx.enter_context(tc.tile_pool(name="xpool", bufs=2))
        hpool = ctx.enter_context(tc.tile_pool(name="hpool", bufs=2))
        qkpool = ctx.enter_context(tc.tile_pool(name="qkpool", bufs=2))
        vpool = ctx.enter_context(tc.tile_pool(name="vpool", bufs=2 * ST))
        ppool = ctx.enter_context(tc.tile_pool(name="ppool", bufs=20))
        opool = ctx.enter_context(tc.tile_pool(name="opool", bufs=2))
        apool = ctx.enter_context(tc.tile_pool(name="apool", bufs=2))
        ypool = ctx.enter_context(tc.tile_pool(name="ypool", bufs=2))
        rpool = ctx.enter_context(tc.tile_pool(name="rpool", bufs=2))
        rbpool = ctx.enter_context(tc.tile_pool(name="rbpool", bufs=2))
        tmp = ctx.enter_context(tc.tile_pool(name="tmp", bufs=2))
        psA = ctx.enter_context(tc.tile_pool(name="psA", bufs=3, space="PSUM"))
        psB = ctx.enter_context(tc.tile_pool(name="psB", bufs=2, space="PSUM"))

        # ---- x loads first: they gate group-norm; const DMAs queue
        #      behind them on the sync engine's in-order DMA-issue stream
        x_list = [None, None]

        def emit_x_load(b):
            x_s = xpool.tile([128, CT, T], F32, tag="x", name=f"x{b}")
            for j in range(CT):
                nc.sync.dma_start(
                    out=x_s[:, j, :], in_=x.ap()[b, 128 * j:128 * (j + 1), :]
                )
            x_list[b] = x_s

        emit_x_load(0)
        emit_x_load(1)

        # ---- constants: small ones first (group-norm needs em/gm/nw/nb
        #      early; the big weights queue after on the DMA-issue stream)
        em_s = consts.tile([8, 128], F32)
        nc.sync.dma_start(out=em_s, in_=em.ap())
        gm_s = consts.tile([128, 8], F32)
        nc.sync.dma_start(out=gm_s, in_=gm.ap())
        nw_s = consts.tile([128, CT], F32)
        nc.sync.dma_start(out=nw_s, in_=nw.ap())
        nb_s = consts.tile([128, CT], F32)
        nc.sync.dma_start(out=nb_s, in_=nb.ap())
        bqk_s = consts.tile([128, NP], F32)
        nc.sync.dma_start(out=bqk_s, in_=bqk.ap())
        pb_s = consts.tile([128, CT], F32)
        nc.sync.dma_start(out=pb_s, in_=pb.ap())
        wqk_s = consts.tile([128, CT, 2 * C], BF16)
        nc.sync.dma_start(out=wqk_s, in_=wqk.ap().rearrange("(j p) n -> p j n", p=128))
        wv_s = consts.tile([128, CT, C], BF16)
        nc.sync.dma_start(out=wv_s, in_=wv.ap().rearrange("(j p) n -> p j n", p=128))
        wp_s = consts.tile([128, CT, C], BF16)
        nc.sync.dma_start(out=wp_s, in_=wp.ap().rearrange("(j p) n -> p j n", p=128))
        eps_s = consts.tile([8, 1], F32)
        nc.vector.memset(eps_s, EPS)

        # ---- per-batch prologue: group-norm stats, h (bf16) ----
        h_list = [None, None]

        def emit_gn_h(b):
            x_s = x_list[b]
            gs = psB.tile([8, 8], F32, tag="B", name=f"gs{b}")
            for j in range(CT):
                st = tmp.tile([128, 2, 6], F32, tag="st")
                nc.vector.bn_stats(out=st[:, 0, :], in_=x_s[:, j, 0:512])
                nc.vector.bn_stats(out=st[:, 1, :], in_=x_s[:, j, 512:1024])
                mv = tmp.tile([128, 2], F32, tag="mv")
                nc.vector.bn_aggr(out=mv, in_=st)
                s2 = tmp.tile([128, 2], F32, tag="s2")
                nc.vector.tensor_copy(out=s2[:, 0:1], in_=mv[:, 0:1])
                # E[x^2] = mean*mean + var
                nc.vector.scalar_tensor_tensor(
                    out=s2[:, 1:2], in0=mv[:, 0:1], scalar=mv[:, 0:1],
                    in1=mv[:, 1:2], op0=OP.mult, op1=OP.add,
                )
                nc.tensor.matmul(gs[:, j:j + 1], gm_s, s2[:, 0:1],
                                 start=True, stop=True)
                nc.tensor.matmul(gs[:, 4 + j:5 + j], gm_s, s2[:, 1:2],
                                 start=True, stop=True)

            gsb = tmp.tile([8, 8], F32, tag="gsb")
            nc.vector.tensor_copy(out=gsb, in_=gs)
            msq = tmp.tile([8, 4], F32, tag="msq")
            nc.vector.tensor_mul(out=msq, in0=gsb[:, 0:4], in1=gsb[:, 0:4])
            varg = tmp.tile([8, 4], F32, tag="varg")
            nc.vector.tensor_tensor(out=varg, in0=gsb[:, 4:8], in1=msq,
                                    op=OP.subtract)
            lng = tmp.tile([8, 4], F32, tag="lng")
            nc.scalar.activation(out=lng, in_=varg, func=AF.Ln, bias=eps_s)
            rstd = tmp.tile([8, 4], F32, tag="rstd")
            nc.scalar.activation(out=rstd, in_=lng, func=AF.Exp, scale=-0.5)
            mr = tmp.tile([8, 8], F32, tag="mr")
            nc.vector.tensor_copy(out=mr[:, 0:4], in_=gsb[:, 0:4])
            nc.vector.tensor_copy(out=mr[:, 4:8], in_=rstd)
            mexp = psB.tile([128, 8], F32, tag="B", name=f"mexp{b}")
            nc.tensor.matmul(mexp, em_s, mr, start=True, stop=True)
            scale_c = tmp.tile([128, CT], F32, tag="scale_c")
            nc.vector.tensor_mul(out=scale_c, in0=mexp[:, 4:8], in1=nw_s)
            mscl = tmp.tile([128, CT], F32, tag="mscl")
            nc.vector.tensor_mul(out=mscl, in0=mexp[:, 0:4], in1=scale_c)
            bias_c = tmp.tile([128, CT], F32, tag="bias_c")
            nc.vector.tensor_tensor(out=bias_c, in0=nb_s, in1=mscl,
                                    op=OP.subtract)

            h_s = hpool.tile([128, CT, T], BF16, tag="h", name=f"h{b}")
            for j in range(CT):
                nc.vector.tensor_scalar(
                    out=h_s[:, j, :], in0=x_s[:, j, :],
                    scalar1=scale_c[:, j:j + 1], scalar2=bias_c[:, j:j + 1],
                    op0=OP.mult, op1=OP.add,
                )
            h_list[b] = h_s

        # ---- emission helpers ----
        v_tiles = [[None] * ST for _ in range(BPC)]
        a_tiles = [[None] * CT for _ in range(BPC)]
        qk_tiles = {}

        # v: one [128(s),512(vdims)] psum tile per s-tile; N=512 = one bank.
        def emit_v_group(b, i):
            h_s = h_list[b]
            pv = psB.tile([128, 512], F32, tag="B", name=f"pv{b}_{i}")
            for kc in range(CT):
                nc.tensor.matmul(
                    pv,
                    h_s[:, kc, i * 128:(i + 1) * 128],
                    wv_s[:, kc, :],
                    start=(kc == 0), stop=(kc == CT - 1),
                )
            v_s = vpool.tile([128, NH, HD + 1], BF16, tag="v",
                             name=f"v{b}_{i}")
            nc.vector.memset(v_s[:, :, HD:HD + 1], 1.0)
            nc.vector.tensor_copy(
                out=v_s[:, :, 0:HD],
                in_=pv.rearrange("p (h d) -> p h d", d=HD),
            )
            v_tiles[b][i] = v_s
            if debug and b == 0 and i == 0:
                nc.sync.dma_start(out=dbg_v.ap(), in_=v_s)

        def emit_qk_half(b, p, which, th):
            """One t-half of q or k for pair p: 4 matmuls into a 1-bank
            psA tile + one DVE op into the bf16 q2/k2 tile."""
            h_s = h_list[b]
            off = p * 256 + (0 if which == "q" else 128)
            pq = psA.tile([128, 512], F32, tag="A",
                          name=f"p{which}{b}_{p}_{th}")
            for kc in range(CT):
                nc.tensor.matmul(
                    pq,
                    wqk_s[:, kc, off:off + 128],
                    h_s[:, kc, th * 512:(th + 1) * 512],
                    start=(kc == 0), stop=(kc == CT - 1),
                )
            if (b, p, which) not in qk_tiles:
                qk_tiles[(b, p, which)] = qkpool.tile(
                    [128, T], BF16, tag=which + "2", name=f"{which}2_{b}_{p}")
            t2 = qk_tiles[(b, p, which)]
            if which == "q":
                nc.vector.tensor_scalar_add(
                    out=t2[:, th * 512:(th + 1) * 512], in0=pq,
                    scalar1=bqk_s[:, p:p + 1])
            else:
                nc.vector.tensor_copy(
                    out=t2[:, th * 512:(th + 1) * 512], in_=pq)

        def emit_qk(b, p):
            for th in range(TH):
                emit_qk_half(b, p, "q", th)
            for th in range(TH):
                emit_qk_half(b, p, "k", th)
            if debug and b == 0 and p == 0:
                nc.sync.dma_start(out=dbg_h.ap(), in_=h_list[b])
                nc.sync.dma_start(out=dbg_q2.ap(), in_=qk_tiles[(b, p, "q")])
                nc.sync.dma_start(out=dbg_k2.ap(), in_=qk_tiles[(b, p, "k")])

        def emit_pair_attention(b, p, interleave=(), self_tail=False):
            """S + exp for pair p, with deferred emissions (previous pair's
            O chains, v groups of the other batch, proj pieces) interleaved
            between s-tiles. Returns closures for this pair's O chains.
            self_tail=True (last pair): head 0's O matmuls are interleaved
            into this pair's own s-loop and head 1's follow immediately, so
            nothing is left pending."""
            q2 = qk_tiles[(b, p, "q")]
            k2 = qk_tiles[(b, p, "k")]
            p_tiles = [[None] * ST, [None] * ST]  # per head-half
            inter = list(interleave)
            st_pO = [None, None]
            st_osb = [None]
            for i in range(ST):
                pS0 = psA.tile([128, T], F32, tag="A", name=f"pS{b}_{p}_{i}a")
                pS1 = psA.tile([128, T], F32, tag="A", name=f"pS{b}_{p}_{i}b")
                # alternate row groups (head 0 rows 0-63, head 1 rows 64-127)
                # so consecutive matmuls can overlap in the PE array
                for th in range(TH):
                    nc.tensor.matmul(
                        pS0[:, th * 512:(th + 1) * 512],
                        k2[0:64, i * 128:(i + 1) * 128],
                        q2[0:64, th * 512:(th + 1) * 512],
                        start=True, stop=True,
                    )
                    nc.tensor.matmul(
                        pS1[:, th * 512:(th + 1) * 512],
                        k2[64:128, i * 128:(i + 1) * 128],
                        q2[64:128, th * 512:(th + 1) * 512],
                        start=True, stop=True,
                    )
                P0 = ppool.tile([128, T], BF16, tag="P", name=f"P{b}_{p}_{i}a")
                nc.scalar.activation(out=P0, in_=pS0, func=AF.Exp, scale=0.125)
                P1 = ppool.tile([128, T], BF16, tag="P", name=f"P{b}_{p}_{i}b")
                nc.scalar.activation(out=P1, in_=pS1, func=AF.Exp, scale=0.125)
                p_tiles[0][i] = P0
                p_tiles[1][i] = P1
                if debug and b == 0 and p == 0 and i == 0:
                    nc.sync.dma_start(out=dbg_P.ap()[0], in_=P0)
                    nc.sync.dma_start(out=dbg_P.ap()[1], in_=P1)
                # one deferred emission per s-tile keeps their waits spread
                # across the exp stream instead of bunching at the boundary
                if self_tail:
                    # drain ALL deferred psB users before the self-tail's
                    # persistent pO tiles take both psB slots (else deadlock)
                    npop = -(-len(inter) // max(1, 4 - i)) if i < 4 else len(inter)
                    for _ in range(npop):
                        inter.pop(0)()
                    if i >= 4:
                        ii = i - 4
                        if ii == 0:
                            st_osb[0] = opool.tile(
                                [HD + 1, T], F32, tag="o", name=f"o{b}_{2 * p}")
                            for th in range(TH):
                                st_pO[th] = psB.tile(
                                    [HD + 1, 512], F32, tag="B",
                                    name=f"pO{b}_{2 * p}_{th}")
                        for th in range(TH):
                            nc.tensor.matmul(
                                st_pO[th],
                                v_tiles[b][ii][:, 2 * p, :],
                                p_tiles[0][ii][:, th * 512:(th + 1) * 512],
                                start=(ii == 0), stop=(ii == ST - 1),
                            )
                else:
                    # front-load deferred emissions (2/s-tile early on) so
                    # s-tile 7 and the pair boundary stay clear of them
                    npop = 2 if i < 5 else (1 if i < ST - 1 else 0)
                    for _ in range(min(npop, len(inter))):
                        inter.pop(0)()
            for e in inter:
                e()

            def make_o_group(half, th, chain=True):
                def emit():
                    j = 2 * p + half
                    o_sb = o_sbs[half]
                    pO = psB.tile([HD + 1, 512], F32, tag="B",
                                  name=f"pO{b}_{j}_{th}")
                    for i in range(ST):
                        nc.tensor.matmul(
                            pO,
                            v_tiles[b][i][:, j, :],
                            p_tiles[half][i][:, th * 512:(th + 1) * 512],
                            start=(i == 0), stop=(i == ST - 1),
                        )
                    nc.vector.tensor_copy(
                        out=o_sb[:, th * 512:(th + 1) * 512], in_=pO)
                    if chain and th == TH - 1:
                        emit_z_chain(half, o_sb)
                return emit

            o_sbs = [None, None]

            def alloc_osb():
                for half in range(2):
                    o_sbs[half] = opool.tile(
                        [HD + 1, T], F32, tag="o", name=f"o{b}_{2 * p + half}")

            def emit_z_chain(half, o_sb):
                j = 2 * p + half
                # partition_broadcast / custom-DVE ops read the tile's
                # absolute partition 0 (AP base offsets ignored) -- move the
                # Z row to a base-0 tile via DMA first.
                z0 = rpool.tile([1, T], F32, tag="z0", name=f"z0{b}_{j}")
                nc.sync.dma_start(out=z0, in_=o_sb[HD:HD + 1, :])
                r_s = rpool.tile([1, T], F32, tag="r", name=f"r{b}_{j}")
                nc.vector.reciprocal_approx_fast(out=r_s, in_=z0)
                rb_s = rbpool.tile([HD, T], F32, tag="rb", name=f"rb{b}_{j}")
                nc.gpsimd.partition_broadcast(out_ap=rb_s, in_ap=r_s)
                if debug and b == 0 and j < 2:
                    nc.sync.dma_start(out=dbg_o.ap()[j], in_=o_sb)
                    nc.sync.dma_start(out=dbg_r.ap()[j], in_=r_s)
                if a_tiles[b][j // 2] is None:
                    a_tiles[b][j // 2] = apool.tile(
                        [128, T], BF16, tag=f"a{j // 2}", name=f"a{b}_{j // 2}")
                po2 = (j % 2) * 64
                nc.vector.tensor_mul(
                    out=a_tiles[b][j // 2][po2:po2 + 64, :],
                    in0=o_sb[0:HD, :], in1=rb_s,
                )

            if self_tail:
                # finish head 0: remaining accumulation steps + copies
                for ii in range(4, ST):
                    for th in range(TH):
                        nc.tensor.matmul(
                            st_pO[th],
                            v_tiles[b][ii][:, 2 * p, :],
                            p_tiles[0][ii][:, th * 512:(th + 1) * 512],
                            start=(ii == 0), stop=(ii == ST - 1),
                        )
                o_sbs[0] = st_osb[0]
                for th in range(TH):
                    nc.vector.tensor_copy(
                        out=st_osb[0][:, th * 512:(th + 1) * 512],
                        in_=st_pO[th])
                # tail: run the two heads' Z chains with maximal overlap,
                # and start this batch's proj (kc 0..2) under them.
                z0a = rpool.tile([1, T], F32, tag="z0", name=f"z0{b}_{2 * p}")
                nc.sync.dma_start(out=z0a, in_=st_osb[0][HD:HD + 1, :])
                ra = rpool.tile([1, T], F32, tag="r", name=f"r{b}_{2 * p}")
                nc.vector.reciprocal_approx_fast(out=ra, in_=z0a)
                o_sbs[1] = opool.tile([HD + 1, T], F32, tag="o",
                                      name=f"o{b}_{2 * p + 1}")
                for th in range(TH):
                    make_o_group(1, th, chain=False)()
                for jo in range(CT - 1):
                    emit_proj_start(b, jo, 3)
                z0b = rpool.tile([1, T], F32, tag="z0",
                                 name=f"z0{b}_{2 * p + 1}")
                nc.sync.dma_start(out=z0b, in_=o_sbs[1][HD:HD + 1, :])
                rb_ = rpool.tile([1, T], F32, tag="r", name=f"r{b}_{2 * p + 1}")
                nc.vector.reciprocal_approx_fast(out=rb_, in_=z0b)
                rba = rbpool.tile([HD, T], F32, tag="rb", name=f"rb{b}_{2 * p}")
                nc.gpsimd.partition_broadcast(out_ap=rba, in_ap=ra)
                rbb = rbpool.tile([HD, T], F32, tag="rb",
                                  name=f"rb{b}_{2 * p + 1}")
                nc.gpsimd.partition_broadcast(out_ap=rbb, in_ap=rb_)
                if a_tiles[b][p] is None:
                    a_tiles[b][p] = apool.tile(
                        [128, T], BF16, tag=f"a{p}", name=f"a{b}_{p}")
                nc.vector.tensor_mul(out=a_tiles[b][p][0:HD, :],
                                     in0=o_sbs[0][0:HD, :], in1=rba)
                nc.vector.tensor_mul(out=a_tiles[b][p][HD:128, :],
                                     in0=o_sbs[1][0:HD, :], in1=rbb)
                for jo in range(CT - 1):
                    emit_proj_finish(b, jo, 3)
                emit_proj(b, CT - 1)
                return []

            groups = [alloc_osb]
            for half in range(2):
                for th in range(TH):
                    groups.append(make_o_group(half, th))
            return groups

        pp_tiles = {}

        def emit_proj_start(b, jo, nkc):
            for th in range(TH):
                pp = psA.tile([128, 512], F32, tag="A", name=f"pp{b}_{jo}_{th}")
                pp_tiles[(b, jo, th)] = pp
                for kc in range(nkc):
                    nc.tensor.matmul(
                        pp,
                        wp_s[:, kc, jo * 128:(jo + 1) * 128],
                        a_tiles[b][kc][:, th * 512:(th + 1) * 512],
                        start=(kc == 0), stop=(kc == CT - 1),
                    )

        def emit_proj_finish(b, jo, nkc):
            y_s = ypool.tile([128, T], F32, tag="y", name=f"y{b}_{jo}")
            for th in range(TH):
                pp = pp_tiles[(b, jo, th)]
                for kc in range(nkc, CT):
                    nc.tensor.matmul(
                        pp,
                        wp_s[:, kc, jo * 128:(jo + 1) * 128],
                        a_tiles[b][kc][:, th * 512:(th + 1) * 512],
                        start=(kc == 0), stop=(kc == CT - 1),
                    )
                nc.vector.scalar_tensor_tensor(
                    out=y_s[:, th * 512:(th + 1) * 512], in0=pp,
                    scalar=pb_s[:, jo:jo + 1],
                    in1=x_list[b][:, jo, th * 512:(th + 1) * 512],
                    op0=OP.add, op1=OP.add,
                )
            nc.sync.dma_start(
                out=y.ap()[b, 128 * jo:128 * (jo + 1), :], in_=y_s
            )

        def emit_proj(b, jo):
            if debug and b == 0 and jo == 0:
                for kc in range(CT):
                    nc.sync.dma_start(out=dbg_a.ap()[:, kc, :],
                                      in_=a_tiles[0][kc])
            emit_proj_start(b, jo, CT)
            emit_proj_finish(b, jo, CT)

        # ---- schedule ----
        # head: batch-0 gn/h -> first qk immediately (v groups and batch-1
        # gn/h ride behind it). Each pair's O chains AND the next pair's
        # qk generation are interleaved into the s-loop so pair boundaries
        # have no serial block; the last pair self-interleaves its O work
        # and proj so nothing trails but the final y writes.
        emit_gn_h(0)
        emit_qk(0, 0)
        emit_gn_h(1)
        for i in range(ST):
            emit_v_group(0, i)
        vb1 = [(lambda b=1, i=i: emit_v_group(b, i)) for i in range(ST)]

        def weave(pend_, qks, extras):
            # alternate previous-pair O groups with next-pair qk halves;
            # the front-loaded pop schedule drains all of it by s-tile 6.
            out, a, q = [], list(pend_), list(qks)
            while a or q:
                if a:
                    out.append(a.pop(0))
                if q:
                    out.append(q.pop(0))
            return out + list(extras)

        def qk_closures(b, p):
            return [(lambda b_=b, p_=p, w_=w, t_=t: emit_qk_half(b_, p_, w_, t_))
                    for w in ("q", "k") for t in range(TH)]

        pend = []
        for p in range(NP):
            qks = qk_closures(0, p + 1) if p < NP - 1 else qk_closures(1, 0)
            extras = (vb1[0:3] if p == 1 else vb1[3:6] if p == 2
                      else vb1[6:8] if p == 3 else [])
            pend = emit_pair_attention(0, p,
                                       interleave=weave(pend, qks, extras))
        # batch 1 pairs with batch-0 proj interleaved
        for p in range(NP):
            qks = qk_closures(1, p + 1) if p < NP - 1 else []
            extras = [(lambda jo=p: emit_proj(0, jo))]
            pend = emit_pair_attention(1, p,
                                       interleave=weave(pend, qks, extras),
                                       self_tail=(p == NP - 1))

    nc.finalize()
    return nc


def _prepack(qkv_w, qkv_b, proj_w, proj_b, norm_w, norm_b):
    """Host-side weight packing (numpy; matmul operands cast to bf16)."""
    import ml_dtypes

    BF = ml_dtypes.bfloat16
    wqk = np.empty((C, 2 * C), dtype=np.float32)
    bqk = np.empty((128, NP), dtype=np.float32)
    wv = np.empty((C, C), dtype=np.float32)
    bv = np.empty((C,), dtype=np.float32)
    for h in range(NH):
        base = 3 * HD * h  # 192h
        p, half = h // 2, h % 2
        # pair-major: [q(2p)|q(2p+1)] then [k(2p)|k(2p+1)], 256 cols/pair
        wqk[:, 256 * p + 64 * half: 256 * p + 64 * (half + 1)] = \
            qkv_w[base:base + HD, :].T
        wqk[:, 256 * p + 128 + 64 * half: 256 * p + 128 + 64 * (half + 1)] = \
            qkv_w[base + HD:base + 2 * HD, :].T
        bqk[64 * half:64 * (half + 1), p] = qkv_b[base:base + HD]
        wv[:, HD * h:HD * (h + 1)] = qkv_w[base + 128:base + 192, :].T
        bv[HD * h:HD * (h + 1)] = qkv_b[base + 128:base + 192]
    wp = np.ascontiguousarray(proj_w.T)
    pbv = proj_b + proj_w @ bv
    pb = np.ascontiguousarray(pbv.reshape(CT, 128).T)
    nw = np.ascontiguousarray(norm_w.reshape(CT, 128).T)
    nb = np.ascontiguousarray(norm_b.reshape(CT, 128).T)
    em = np.zeros((8, 128), dtype=np.float32)
    gm = np.zeros((128, 8), dtype=np.float32)
    for p in range(128):
        em[p // 16, p] = 1.0
        gm[p, p // 16] = 1.0 / 16.0  # bn_aggr outputs are already per-T means
    return dict(wqk=wqk.astype(BF), bqk=bqk, wv=wv.astype(BF),
                wp=wp.astype(BF), pb=pb, nw=nw, nb=nb, em=em, gm=gm)


def kernel(**inputs):
    from concourse.bass_utils import run_bass_kernel_spmd

    x = np.ascontiguousarray(np.asarray(inputs["x"], dtype=np.float32))
    assert x.shape == (B, C, 32, 32)
    nh = int(np.asarray(inputs["num_heads"]))
    assert nh == NH, f"kernel hardcodes num_heads={NH}, got {nh}"

    packed = _prepack(
        np.asarray(inputs["qkv_w"], dtype=np.float32),
        np.asarray(inputs["qkv_b"], dtype=np.float32),
        np.asarray(inputs["proj_w"], dtype=np.float32),
        np.asarray(inputs["proj_b"], dtype=np.float32),
        np.asarray(inputs["norm_w"], dtype=np.float32),
        np.asarray(inputs["norm_b"], dtype=np.float32),
    )

    if "nc" not in _CACHE:
        _CACHE["nc"] = _build_nc()
    nc = _CACHE["nc"]

    xr = x.reshape(B, C, T)
    in_maps = []
    for c in range(NCORES):
        m = dict(packed)
        m["x"] = np.ascontiguousarray(xr[c * BPC:(c + 1) * BPC])
        in_maps.append(m)

    # Execute twice and compare: guards against a rare first-execution
    # flake observed after a fresh NEFF load. Extra exec costs ~ms.
    def run_once():
        res = run_bass_kernel_spmd(nc, in_maps, core_ids=list(range(NCORES)))
        return np.concatenate(
            [res.results[c]["y"] for c in range(NCORES)], axis=0
        )

    out1 = run_once()
    out2 = run_once()
    if not np.array_equal(out1, out2):
        out3 = run_once()
        out1 = out3 if np.array_equal(out2, out3) else out2
        if np.array_equal(out2, out3):
            out1 = out2
    return out1.reshape(B, C, 32, 32).astype(np.float32)


# revision 50
# speedup vs baseline: 1.0994x; 1.0536x over previous
"""Trainium2 Bass kernel for nn_Attention (GroupNorm + MHA + proj + residual).

Reference (per batch b of 16, C=512, T=32*32=1024, 8 heads, head_dim 64):
  xr   = x.reshape(B, C, T)
  h    = group_norm(xr, 32 groups of 16 ch x T)  * norm_w + norm_b
  qkv  = qkv_w @ h + qkv_b          (per-head contiguous [q;k;v] chunks)
  S    = (q/8^.5)^T (k/8^.5)        per head-batch  [T, T]
  P    = softmax(S)
  o    = P @ v^T  -> [ch, T];  out = proj_w @ o + proj_b + xr

Sharding: pure data-parallel over batch: 2 batches per core x 8 cores.

v2 design (vs baseline):
  - all matmul operands bf16 (f32r weight loads kept the PE array at ~50%
    duty -> HAM clock-gate throttled it to 1.2GHz for half the kernel)
  - S matmuls row-paired: heads (2p, 2p+1) live in partitions 0-63/64-127
    of shared q2/k2 tiles; the two K=64 matmuls run concurrently
  - k-bias dropped (adds a per-query constant to logits -> cancels in
    softmax); q-bias applied as one per-pair DVE op
  - per-head Z chain: o_sb copy -> reciprocal_approx_fast on the Z row
    -> gpsimd partition_broadcast -> gpsimd multiply (no SBUF-SBUF DMAs)
  - PSUM: pool A 3 x [128,1024] (qk / S / proj rotation)
          pool B 2 x [128,512]  (v tiles / O accumulation per t-half)
"""
import math
import numpy as np

B, C, T, NH, HD = 16, 512, 1024, 8, 64
NCORES = 8
BPC = B // NCORES          # batches per core
CT = C // 128              # channel tiles (4)
ST = T // 128              # s tiles (8)
TH = T // 512              # t halves (2)
NP = NH // 2               # head pairs (4)
EPS = 1e-5

_CACHE = {}


def _build_nc(debug=False):
    import concourse.bass as bass
    from concourse import bacc
    import concourse.tile as tile
    from concourse import mybir
    from contextlib import ExitStack

    F32 = mybir.dt.float32
    BF16 = mybir.dt.bfloat16
    AF = mybir.ActivationFunctionType
    OP = mybir.AluOpType

    nc = bacc.Bacc(trn_type="TRN2", name="attn")

    x = nc.dram_tensor("x", [BPC, C, T], F32, kind="ExternalInput")
    wqk = nc.dram_tensor("wqk", [C, 2 * C], BF16, kind="ExternalInput")
    bqk = nc.dram_tensor("bqk", [128, NP], F32, kind="ExternalInput")
    wv = nc.dram_tensor("wv", [C, C], BF16, kind="ExternalInput")
    wp = nc.dram_tensor("wp", [C, C], BF16, kind="ExternalInput")
    pb = nc.dram_tensor("pb", [128, CT], F32, kind="ExternalInput")
    nw = nc.dram_tensor("nw", [128, CT], F32, kind="ExternalInput")
    nb = nc.dram_tensor("nb", [128, CT], F32, kind="ExternalInput")
    em = nc.dram_tensor("em", [8, 128], F32, kind="ExternalInput")
    gm = nc.dram_tensor("gm", [128, 8], F32, kind="ExternalInput")
    y = nc.dram_tensor("y", [BPC, C, T], F32, kind="ExternalOutput")
    if debug:
        dbg_h = nc.dram_tensor("dbg_h", [128, CT, T], BF16, kind="ExternalOutput")
        dbg_q2 = nc.dram_tensor("dbg_q2", [128, T], BF16, kind="ExternalOutput")
        dbg_k2 = nc.dram_tensor("dbg_k2", [128, T], BF16, kind="ExternalOutput")
        dbg_v = nc.dram_tensor("dbg_v", [128, NH, HD + 1], BF16, kind="ExternalOutput")
        dbg_P = nc.dram_tensor("dbg_P", [2, 128, T], BF16, kind="ExternalOutput")
        dbg_o = nc.dram_tensor("dbg_o", [2, HD + 1, T], F32, kind="ExternalOutput")
        dbg_r = nc.dram_tensor("dbg_r", [2, 1, T], F32, kind="ExternalOutput")
        dbg_a = nc.dram_tensor("dbg_a", [128, CT, T], BF16, kind="ExternalOutput")

    with tile.TileContext(nc) as tc, ExitStack() as ctx:
        consts = ctx.enter_context(tc.tile_pool(name="consts", bufs=1))
        xpool = ctx.enter_context(tc.tile_pool(name="xpool", bufs=2))
        hpool = ctx.enter_context(tc.tile_pool(name="hpool", bufs=2))
        qkpool = ctx.enter_context(tc.tile_pool(name="qkpool", bufs=2))
        vpool = ctx.enter_context(tc.tile_pool(name="vpool", bufs=2 * ST))
        ppool = ctx.enter_context(tc.tile_pool(name="ppool", bufs=24))
        opool = ctx.enter_context(tc.tile_pool(name="opool", bufs=2))
        apool = ctx.enter_context(tc.tile_pool(name="apool", bufs=2))
        ypool = ctx.enter_context(tc.tile_pool(name="ypool", bufs=2))
        rpool = ctx.enter_context(tc.tile_pool(name="rpool", bufs=2))
        rbpool = ctx.enter_context(tc.tile_pool(name="rbpool", bufs=2))
        tmp = ctx.enter_context(tc.tile_pool(name="tmp", bufs=2))
        psA = ctx.enter_context(tc.tile_pool(name="psA", bufs=3, space="PSUM"))
        psB = ctx.enter_context(tc.tile_pool(name="psB", bufs=2, space="PSUM"))

        # ---- x loads first: they gate group-norm; const DMAs queue
        #      behind them on the sync engine's in-order DMA-issue stream
        x_list = [None, None]

        def emit_x_load(b):
            x_s = xpool.tile([128, CT, T], F32, tag="x", name=f"x{b}")
            for j in range(CT):
                nc.sync.dma_start(
                    out=x_s[:, j, :], in_=x.ap()[b, 128 * j:128 * (j + 1), :]
                )
            x_list[b] = x_s

        emit_x_load(0)
        emit_x_load(1)

        # ---- constants: small ones first (group-norm needs em/gm/nw/nb
        #      early; the big weights queue after on the DMA-issue stream)
        em_s = consts.tile([8, 128], F32)
        nc.sync.dma_start(out=em_s, in_=em.ap())
        gm_s = consts.tile([128, 8], F32)
        nc.sync.dma_start(out=gm_s, in_=gm.ap())
        nw_s = consts.tile([128, CT], F32)
        nc.sync.dma_start(out=nw_s, in_=nw.ap())
        nb_s = consts.tile([128, CT], F32)
        nc.sync.dma_start(out=nb_s, in_=nb.ap())
        bqk_s = consts.tile([128, NP], F32)
        nc.sync.dma_start(out=bqk_s, in_=bqk.ap())
        pb_s = consts.tile([128, CT], F32)
        nc.sync.dma_start(out=pb_s, in_=pb.ap())
        wqk_s = consts.tile([128, CT, 2 * C], BF16)
        nc.sync.dma_start(out=wqk_s, in_=wqk.ap().rearrange("(j p) n -> p j n", p=128))
        wv_s = consts.tile([128, CT, C], BF16)
        nc.sync.dma_start(out=wv_s, in_=wv.ap().rearrange("(j p) n -> p j n", p=128))
        wp_s = consts.tile([128, CT, C], BF16)
        nc.sync.dma_start(out=wp_s, in_=wp.ap().rearrange("(j p) n -> p j n", p=128))
        eps_s = consts.tile([8, 1], F32)
        nc.vector.memset(eps_s, EPS)

        # ---- per-batch prologue: group-norm stats, h (bf16) ----
        h_list = [None, None]

        def emit_gn_h(b):
            x_s = x_list[b]
            gs = psA.tile([8, 8], F32, tag="A", name=f"gs{b}")
            for j in range(CT):
                st = tmp.tile([128, 2, 6], F32, tag="st")
                nc.vector.bn_stats(out=st[:, 0, :], in_=x_s[:, j, 0:512])
                nc.vector.bn_stats(out=st[:, 1, :], in_=x_s[:, j, 512:1024])
                mv = tmp.tile([128, 2], F32, tag="mv")
                nc.vector.bn_aggr(out=mv, in_=st)
                s2 = tmp.tile([128, 2], F32, tag="s2")
                nc.vector.tensor_copy(out=s2[:, 0:1], in_=mv[:, 0:1])
                # E[x^2] = mean*mean + var
                nc.vector.scalar_tensor_tensor(
                    out=s2[:, 1:2], in0=mv[:, 0:1], scalar=mv[:, 0:1],
                    in1=mv[:, 1:2], op0=OP.mult, op1=OP.add,
                )
                nc.tensor.matmul(gs[:, j:j + 1], gm_s, s2[:, 0:1],
                                 start=True, stop=True)
                nc.tensor.matmul(gs[:, 4 + j:5 + j], gm_s, s2[:, 1:2],
                                 start=True, stop=True)

            gsb = tmp.tile([8, 8], F32, tag="gsb")
            nc.vector.tensor_copy(out=gsb, in_=gs)
            msq = tmp.tile([8, 4], F32, tag="msq")
            nc.vector.tensor_mul(out=msq, in0=gsb[:, 0:4], in1=gsb[:, 0:4])
            varg = tmp.tile([8, 4], F32, tag="varg")
            nc.vector.tensor_tensor(out=varg, in0=gsb[:, 4:8], in1=msq,
                                    op=OP.subtract)
            lng = tmp.tile([8, 4], F32, tag="lng")
            nc.scalar.activation(out=lng, in_=varg, func=AF.Ln, bias=eps_s)
            rstd = tmp.tile([8, 4], F32, tag="rstd")
            nc.scalar.activation(out=rstd, in_=lng, func=AF.Exp, scale=-0.5)
            mr = tmp.tile([8, 8], F32, tag="mr")
            nc.vector.tensor_copy(out=mr[:, 0:4], in_=gsb[:, 0:4])
            nc.vector.tensor_copy(out=mr[:, 4:8], in_=rstd)
            mexp = psA.tile([128, 8], F32, tag="A", name=f"mexp{b}")
            nc.tensor.matmul(mexp, em_s, mr, start=True, stop=True)
            scale_c = tmp.tile([128, CT], F32, tag="scale_c")
            nc.vector.tensor_mul(out=scale_c, in0=mexp[:, 4:8], in1=nw_s)
            mscl = tmp.tile([128, CT], F32, tag="mscl")
            nc.vector.tensor_mul(out=mscl, in0=mexp[:, 0:4], in1=scale_c)
            bias_c = tmp.tile([128, CT], F32, tag="bias_c")
            nc.vector.tensor_tensor(out=bias_c, in0=nb_s, in1=mscl,
                                    op=OP.subtract)

            h_s = hpool.tile([128, CT, T], BF16, tag="h", name=f"h{b}")
            for j in range(CT):
                nc.vector.tensor_scalar(
                    out=h_s[:, j, :], in0=x_s[:, j, :],
                    scalar1=scale_c[:, j:j + 1], scalar2=bias_c[:, j:j + 1],
                    op0=OP.mult, op1=OP.add,
                )
            h_list[b] = h_s

        # ---- emission helpers ----
        v_tiles = [[None] * ST for _ in range(BPC)]
        a_tiles = [[None] * CT for _ in range(BPC)]
        qk_tiles = {}

        # v: one [128(s),512(vdims)] psum tile per s-tile; N=512 = one bank.
        def emit_v_group(b, i):
            h_s = h_list[b]
            pv = psB.tile([128, 512], F32, tag="B", name=f"pv{b}_{i}")
            for kc in range(CT):
                nc.tensor.matmul(
                    pv,
                    h_s[:, kc, i * 128:(i + 1) * 128],
                    wv_s[:, kc, :],
                    start=(kc == 0), stop=(kc == CT - 1),
                )
            v_s = vpool.tile([128, NH, HD + 1], BF16, tag="v",
                             name=f"v{b}_{i}")
            nc.vector.memset(v_s[:, :, HD:HD + 1], 1.0)
            nc.vector.tensor_copy(
                out=v_s[:, :, 0:HD],
                in_=pv.rearrange("p (h d) -> p h d", d=HD),
            )
            v_tiles[b][i] = v_s
            if debug and b == 0 and i == 0:
                nc.sync.dma_start(out=dbg_v.ap(), in_=v_s)

        def emit_qk_q(b, p):
            h_s = h_list[b]
            pqq = psA.tile([128, T], F32, tag="A", name=f"pqq{b}_{p}")
            for th in range(TH):
                for kc in range(CT):
                    nc.tensor.matmul(
                        pqq[:, th * 512:(th + 1) * 512],
                        wqk_s[:, kc, p * 256:p * 256 + 128],
                        h_s[:, kc, th * 512:(th + 1) * 512],
                        start=(kc == 0), stop=(kc == CT - 1),
                    )
            q2 = qkpool.tile([128, T], BF16, tag="q2", name=f"q2_{b}_{p}")
            nc.vector.tensor_scalar_add(out=q2, in0=pqq,
                                        scalar1=bqk_s[:, p:p + 1])
            qk_tiles[(b, p, "q")] = q2

        def emit_qk_k(b, p):
            h_s = h_list[b]
            pqk = psA.tile([128, T], F32, tag="A", name=f"pqk{b}_{p}")
            for th in range(TH):
                for kc in range(CT):
                    nc.tensor.matmul(
                        pqk[:, th * 512:(th + 1) * 512],
                        wqk_s[:, kc, p * 256 + 128:(p + 1) * 256],
                        h_s[:, kc, th * 512:(th + 1) * 512],
                        start=(kc == 0), stop=(kc == CT - 1),
                    )
            k2 = qkpool.tile([128, T], BF16, tag="k2", name=f"k2_{b}_{p}")
            nc.vector.tensor_copy(out=k2, in_=pqk)
            qk_tiles[(b, p, "k")] = k2
            if debug and b == 0 and p == 0:
                nc.sync.dma_start(out=dbg_h.ap(), in_=h_s)
                nc.sync.dma_start(out=dbg_q2.ap(), in_=qk_tiles[(b, p, "q")])
                nc.sync.dma_start(out=dbg_k2.ap(), in_=k2)

        def emit_qk(b, p):
            emit_qk_q(b, p)
            emit_qk_k(b, p)

        def emit_pair_attention(b, p, interleave=(), self_tail=False):
            """S + exp for pair p, with deferred emissions (previous pair's
            O chains, v groups of the other batch, proj pieces) interleaved
            between s-tiles. Returns closures for this pair's O chains.
            self_tail=True (last pair): head 0's O matmuls are interleaved
            into this pair's own s-loop and head 1's follow immediately, so
            nothing is left pending."""
            q2 = qk_tiles[(b, p, "q")]
            k2 = qk_tiles[(b, p, "k")]
            p_tiles = [[None] * ST, [None] * ST]  # per head-half
            inter = list(interleave)
            st_pO = [None, None]
            st_osb = [None]
            for i in range(ST):
                pS0 = psA.tile([128, T], F32, tag="A", name=f"pS{b}_{p}_{i}a")
                pS1 = psA.tile([128, T], F32, tag="A", name=f"pS{b}_{p}_{i}b")
                # alternate row groups (head 0 rows 0-63, head 1 rows 64-127)
                # so consecutive matmuls overlap in the PE array
                for th in range(TH):
                    nc.tensor.matmul(
                        pS0[:, th * 512:(th + 1) * 512],
                        k2[0:64, i * 128:(i + 1) * 128],
                        q2[0:64, th * 512:(th + 1) * 512],
                        start=True, stop=True,
                    )
                    nc.tensor.matmul(
                        pS1[:, th * 512:(th + 1) * 512],
                        k2[64:128, i * 128:(i + 1) * 128],
                        q2[64:128, th * 512:(th + 1) * 512],
                        start=True, stop=True,
                    )
                P0 = ppool.tile([128, T], BF16, tag="P", name=f"P{b}_{p}_{i}a")
                nc.scalar.activation(out=P0, in_=pS0, func=AF.Exp, scale=0.125)
                P1 = ppool.tile([128, T], BF16, tag="P", name=f"P{b}_{p}_{i}b")
                nc.scalar.activation(out=P1, in_=pS1, func=AF.Exp, scale=0.125)
                p_tiles[0][i] = P0
                p_tiles[1][i] = P1
                if debug and b == 0 and p == 0 and i == 0:
                    nc.sync.dma_start(out=dbg_P.ap()[0], in_=P0)
                    nc.sync.dma_start(out=dbg_P.ap()[1], in_=P1)
                # one deferred emission per s-tile keeps their waits spread
                # across the exp stream instead of bunching at the boundary
                if self_tail:
                    # drain ALL deferred psB users before the self-tail's
                    # persistent pO tiles take both psB slots (else deadlock)
                    npop = -(-len(inter) // max(1, 4 - i)) if i < 4 else len(inter)
                    for _ in range(npop):
                        inter.pop(0)()
                    if i >= 4:
                        ii = i - 4
                        if ii == 0:
                            st_osb[0] = opool.tile(
                                [HD + 1, T], F32, tag="o", name=f"o{b}_{2 * p}")
                            for th in range(TH):
                                st_pO[th] = psB.tile(
                                    [HD + 1, 512], F32, tag="B",
                                    name=f"pO{b}_{2 * p}_{th}")
                        for th in range(TH):
                            nc.tensor.matmul(
                                st_pO[th],
                                v_tiles[b][ii][:, 2 * p, :],
                                p_tiles[0][ii][:, th * 512:(th + 1) * 512],
                                start=(ii == 0), stop=(ii == ST - 1),
                            )
                else:
                    # front-load deferred emissions (2/s-tile early on) so
                    # s-tile 7 and the pair boundary stay clear of them
                    npop = 2 if i < 3 else (1 if i < ST - 1 else 0)
                    for _ in range(min(npop, len(inter))):
                        inter.pop(0)()
            for e in inter:
                e()

            def make_o_group(half, th, chain=True, zdma=None):
                def emit():
                    j = 2 * p + half
                    o_sb = o_sbs[half]
                    pO = psB.tile([HD + 1, 512], F32, tag="B",
                                  name=f"pO{b}_{j}_{th}")
                    for i in range(ST):
                        nc.tensor.matmul(
                            pO,
                            v_tiles[b][i][:, j, :],
                            p_tiles[half][i][:, th * 512:(th + 1) * 512],
                            start=(i == 0), stop=(i == ST - 1),
                        )
                    nc.vector.tensor_copy(
                        out=o_sb[:, th * 512:(th + 1) * 512], in_=pO)
                    if zdma is not None:
                        # per-half Z extraction right after the copy so the
                        # reciprocal chain starts before the second half
                        nc.sync.dma_start(
                            out=zdma[:, th * 512:(th + 1) * 512],
                            in_=o_sb[HD:HD + 1, th * 512:(th + 1) * 512])
                    if chain and th == TH - 1:
                        emit_z_chain(half, o_sb)
                return emit

            o_sbs = [None, None]

            def alloc_osb():
                for half in range(2):
                    o_sbs[half] = opool.tile(
                        [HD + 1, T], F32, tag="o", name=f"o{b}_{2 * p + half}")

            def emit_z_chain(half, o_sb):
                j = 2 * p + half
                # partition_broadcast / custom-DVE ops read the tile's
                # absolute partition 0 (AP base offsets ignored) -- move the
                # Z row to a base-0 tile via DMA first.
                z0 = rpool.tile([1, T], F32, tag="z0", name=f"z0{b}_{j}")
                nc.sync.dma_start(out=z0, in_=o_sb[HD:HD + 1, :])
                r_s = rpool.tile([1, T], F32, tag="r", name=f"r{b}_{j}")
                nc.vector.reciprocal_approx_fast(out=r_s, in_=z0)
                rb_s = rbpool.tile([HD, T], F32, tag="rb", name=f"rb{b}_{j}")
                nc.gpsimd.partition_broadcast(out_ap=rb_s, in_ap=r_s)
                if debug and b == 0 and j < 2:
                    nc.sync.dma_start(out=dbg_o.ap()[j], in_=o_sb)
                    nc.sync.dma_start(out=dbg_r.ap()[j], in_=r_s)
                if a_tiles[b][j // 2] is None:
                    a_tiles[b][j // 2] = apool.tile(
                        [128, T], BF16, tag=f"a{j // 2}", name=f"a{b}_{j // 2}")
                po2 = (j % 2) * 64
                nc.vector.tensor_mul(
                    out=a_tiles[b][j // 2][po2:po2 + 64, :],
                    in0=o_sb[0:HD, :], in1=rb_s,
                )

            if self_tail:
                # finish head 0: remaining accumulation steps + copies
                for ii in range(4, ST):
                    for th in range(TH):
                        nc.tensor.matmul(
                            st_pO[th],
                            v_tiles[b][ii][:, 2 * p, :],
                            p_tiles[0][ii][:, th * 512:(th + 1) * 512],
                            start=(ii == 0), stop=(ii == ST - 1),
                        )
                o_sbs[0] = st_osb[0]
                for th in range(TH):
                    nc.vector.tensor_copy(
                        out=st_osb[0][:, th * 512:(th + 1) * 512],
                        in_=st_pO[th])
                # tail: run the two heads' Z chains with maximal overlap,
                # and start this batch's proj (kc 0..2) under them.
                z0a = rpool.tile([1, T], F32, tag="z0", name=f"z0{b}_{2 * p}")
                nc.sync.dma_start(out=z0a, in_=st_osb[0][HD:HD + 1, :])
                ra = rpool.tile([1, T], F32, tag="r", name=f"r{b}_{2 * p}")
                nc.vector.reciprocal_approx_fast(out=ra, in_=z0a)
                o_sbs[1] = opool.tile([HD + 1, T], F32, tag="o",
                                      name=f"o{b}_{2 * p + 1}")
                z0b = rpool.tile([1, T], F32, tag="z0",
                                 name=f"z0{b}_{2 * p + 1}")
                for th in range(TH):
                    make_o_group(1, th, chain=False, zdma=z0b)()
                for jo in range(CT - 1):
                    emit_proj_start(b, jo, 3)
                rb_ = rpool.tile([1, T], F32, tag="r", name=f"r{b}_{2 * p + 1}")
                nc.vector.reciprocal_approx_fast(out=rb_, in_=z0b)
                rba = rbpool.tile([HD, T], F32, tag="rb", name=f"rb{b}_{2 * p}")
                nc.gpsimd.partition_broadcast(out_ap=rba, in_ap=ra)
                rbb = rbpool.tile([HD, T], F32, tag="rb",
                                  name=f"rb{b}_{2 * p + 1}")
                nc.gpsimd.partition_broadcast(out_ap=rbb, in_ap=rb_)
                if a_tiles[b][p] is None:
                    a_tiles[b][p] = apool.tile(
                        [128, T], BF16, tag=f"a{p}", name=f"a{b}_{p}")
                nc.vector.tensor_mul(out=a_tiles[b][p][0:HD, :],
                                     in0=o_sbs[0][0:HD, :], in1=rba)
                nc.vector.tensor_mul(out=a_tiles[b][p][HD:128, :],
                                     in0=o_sbs[1][0:HD, :], in1=rbb)
                for jo in range(CT - 1):
                    emit_proj_finish(b, jo, 3)
                emit_proj(b, CT - 1)
                return []

            groups = [alloc_osb]
            for half in range(2):
                for th in range(TH):
                    groups.append(make_o_group(half, th))
            return groups

        pp_tiles = {}

        def emit_proj_start(b, jo, nkc):
            pp = psA.tile([128, T], F32, tag="A", name=f"pp{b}_{jo}")
            pp_tiles[(b, jo)] = pp
            for th in range(TH):
                for kc in range(nkc):
                    nc.tensor.matmul(
                        pp[:, th * 512:(th + 1) * 512],
                        wp_s[:, kc, jo * 128:(jo + 1) * 128],
                        a_tiles[b][kc][:, th * 512:(th + 1) * 512],
                        start=(kc == 0), stop=(kc == CT - 1),
                    )

        def emit_proj_finish(b, jo, nkc):
            pp = pp_tiles[(b, jo)]
            for th in range(TH):
                for kc in range(nkc, CT):
                    nc.tensor.matmul(
                        pp[:, th * 512:(th + 1) * 512],
                        wp_s[:, kc, jo * 128:(jo + 1) * 128],
                        a_tiles[b][kc][:, th * 512:(th + 1) * 512],
                        start=(kc == 0), stop=(kc == CT - 1),
                    )
            y_s = ypool.tile([128, T], F32, tag="y", name=f"y{b}_{jo}")
            nc.vector.scalar_tensor_tensor(
                out=y_s, in0=pp, scalar=pb_s[:, jo:jo + 1],
                in1=x_list[b][:, jo, :], op0=OP.add, op1=OP.add,
            )
            nc.sync.dma_start(
                out=y.ap()[b, 128 * jo:128 * (jo + 1), :], in_=y_s
            )

        def emit_proj(b, jo):
            if debug and b == 0 and jo == 0:
                for kc in range(CT):
                    nc.sync.dma_start(out=dbg_a.ap()[:, kc, :],
                                      in_=a_tiles[0][kc])
            emit_proj_start(b, jo, CT)
            emit_proj_finish(b, jo, CT)

        # ---- schedule ----
        # head: batch-0 gn/h -> first qk immediately (v groups and batch-1
        # gn/h ride behind it). Each pair's O chains AND the next pair's
        # qk generation are interleaved into the s-loop so pair boundaries
        # have no serial block; the last pair self-interleaves its O work
        # and proj so nothing trails but the final y writes.
        emit_gn_h(0)
        emit_qk(0, 0)
        emit_gn_h(1)
        for i in range(ST):
            emit_v_group(0, i)
        vb1 = [(lambda b=1, i=i: emit_v_group(b, i)) for i in range(ST)]

        def weave(pend_, qks, extras):
            # positions 0..7 = s-tiles of the next pair's s-loop. Keep the
            # last O group and the k-copy off s-tiles 6-7 so the PE hits the
            # pair boundary with nothing queued between S(7) and S'(0).
            out, items, q = [], list(pend_), list(qks)
            if items:
                out.append(items.pop(0))          # alloc_osb       @0
            if items:
                out.append(items.pop(0))          # og0             @1
            if q:
                out.append(q.pop(0))              # qk_q            @2
            if items:
                out.append(items.pop(0))          # og1             @3
            if q:
                out.append(q.pop(0))              # qk_k            @4
            return out + items + list(extras)     # og2 @5, og3 @6, extras

        pend = []
        for p in range(NP):
            if p < NP - 1:
                qks = [(lambda q=p + 1: emit_qk_q(0, q)),
                       (lambda q=p + 1: emit_qk_k(0, q))]
            else:
                qks = [(lambda: emit_qk_q(1, 0)), (lambda: emit_qk_k(1, 0))]
            extras = (vb1[0:3] if p == 1 else vb1[3:6] if p == 2
                      else vb1[6:8] if p == 3 else [])
            pend = emit_pair_attention(0, p,
                                       interleave=weave(pend, qks, extras))
        # batch 1 pairs with batch-0 proj interleaved
        for p in range(NP):
            qks = ([(lambda q=p + 1: emit_qk_q(1, q)),
                    (lambda q=p + 1: emit_qk_k(1, q))] if p < NP - 1 else [])
            extras = [(lambda jo=p: emit_proj(0, jo))]
            pend = emit_pair_attention(1, p,
                                       interleave=weave(pend, qks, extras),
                                       self_tail=(p == NP - 1))

    nc.finalize()
    return nc


def _prepack(qkv_w, qkv_b, proj_w, proj_b, norm_w, norm_b):
    """Host-side weight packing (numpy; matmul operands cast to bf16)."""
    import ml_dtypes

    BF = ml_dtypes.bfloat16
    wqk = np.empty((C, 2 * C), dtype=np.float32)
    bqk = np.empty((128, NP), dtype=np.float32)
    wv = np.empty((C, C), dtype=np.float32)
    bv = np.empty((C,), dtype=np.float32)
    for h in range(NH):
        base = 3 * HD * h  # 192h
        p, half = h // 2, h % 2
        # pair-major: [q(2p)|q(2p+1)] then [k(2p)|k(2p+1)], 256 cols/pair
        wqk[:, 256 * p + 64 * half: 256 * p + 64 * (half + 1)] = \
            qkv_w[base:base + HD, :].T
        wqk[:, 256 * p + 128 + 64 * half: 256 * p + 128 + 64 * (half + 1)] = \
            qkv_w[base + HD:base + 2 * HD, :].T
        bqk[64 * half:64 * (half + 1), p] = qkv_b[base:base + HD]
        wv[:, HD * h:HD * (h + 1)] = qkv_w[base + 128:base + 192, :].T
        bv[HD * h:HD * (h + 1)] = qkv_b[base + 128:base + 192]
    wp = np.ascontiguousarray(proj_w.T)
    pbv = proj_b + proj_w @ bv
    pb = np.ascontiguousarray(pbv.reshape(CT, 128).T)
    nw = np.ascontiguousarray(norm_w.reshape(CT, 128).T)
    nb = np.ascontiguousarray(norm_b.reshape(CT, 128).T)
    em = np.zeros((8, 128), dtype=np.float32)
    gm = np.zeros((128, 8), dtype=np.float32)
    for p in range(128):
        em[p // 16, p] = 1.0
        gm[p, p // 16] = 1.0 / 16.0  # bn_aggr outputs are already per-T means
    return dict(wqk=wqk.astype(BF), bqk=bqk, wv=wv.astype(BF),
                wp=wp.astype(BF), pb=pb, nw=nw, nb=nb, em=em, gm=gm)


def kernel(**inputs):
    from concourse.bass_utils import run_bass_kernel_spmd

    x = np.ascontiguousarray(np.asarray(inputs["x"], dtype=np.float32))
    assert x.shape == (B, C, 32, 32)
    nh = int(np.asarray(inputs["num_heads"]))
    assert nh == NH, f"kernel hardcodes num_heads={NH}, got {nh}"

    packed = _prepack(
        np.asarray(inputs["qkv_w"], dtype=np.float32),
        np.asarray(inputs["qkv_b"], dtype=np.float32),
        np.asarray(inputs["proj_w"], dtype=np.float32),
        np.asarray(inputs["proj_b"], dtype=np.float32),
        np.asarray(inputs["norm_w"], dtype=np.float32),
        np.asarray(inputs["norm_b"], dtype=np.float32),
    )

    if "nc" not in _CACHE:
        _CACHE["nc"] = _build_nc()
    nc = _CACHE["nc"]

    xr = x.reshape(B, C, T)
    in_maps = []
    for c in range(NCORES):
        m = dict(packed)
        m["x"] = np.ascontiguousarray(xr[c * BPC:(c + 1) * BPC])
        in_maps.append(m)

    # Execute twice and compare: guards against a rare first-execution
    # flake observed after a fresh NEFF load. Extra exec costs ~ms.
    def run_once():
        res = run_bass_kernel_spmd(nc, in_maps, core_ids=list(range(NCORES)))
        return np.concatenate(
            [res.results[c]["y"] for c in range(NCORES)], axis=0
        )

    out1 = run_once()
    out2 = run_once()
    if not np.array_equal(out1, out2):
        out3 = run_once()
        out1 = out3 if np.array_equal(out2, out3) else out2
        if np.array_equal(out2, out3):
            out1 = out2
    return out1.reshape(B, C, 32, 32).astype(np.float32)
